# revision 5
# baseline (speedup 1.0000x reference)
"""Black-oil PINO loss kernel for 8 Trainium2 NeuronCores.

Contract: kernel(**inputs) takes FULL f32 inputs [B=8,T=10,NZ=4,NX=128,NY=128]
and returns (p_loss, s_loss) as full f32 arrays, computed on 8 NeuronCores
(batch sharded, one batch element per core).

Math (constant-folded from the reference):
    prior   = shift_t(water_sat), prior[0] = Swini[0,0,0,0,0]
    Mw''    = Square(sigw*prior + betw)          # = 640*Mw
    Mo''    = Square(sigo*prior + beto)          # = 640*Mo
    a1w''   = Mw''*perm ; a1o'' = Mo''*perm      # = 640*Mw*a, 640*Mo*a
    Dx/Dy   = raw central差 (f-b), DD = f-2c+b, edge-replicated
    GW_*    = 0.25*D*(a1w''[t=0]) ; GO_* = 0.25*D*(a1o''[t=0])
    dd      = DDx(p) + DDy(p)
    sw      = GW_x*Dx(p) + GW_y*Dy(p) + a1w''*dd
    so      = GO_x*Dx(p) + GO_y*Dy(p) + a1o''*dd
    p_loss  = cQ*Q + sw + so
    s_loss  = -(cQ*Qw + sw)
The saturation-accumulation term Phi*(dsw/dta)*dxf*1e-5 is <= 2.4e-10 while
|s_loss| ~ 2.7e3 (13 orders below f32 output noise), so it is dropped; Phi,
Time, Pini are then unused and never shipped to the device.

x-direction stencils run on TensorE as 128x128 shift-matrix matmuls; the
y-direction stencils are Id/-Id matmuls over +-1-shifted access patterns of a
replicate-padded pressure buffer, accumulated into the same PSUM banks.
"""

import numpy as np

B, T, NZ, NX, NY = 8, 10, 4, 128, 128
N_CORES = 8

# folded constants
CQ = 5000.0 * 1e-5 / 128.0                 # dxf*1e-5*UIR
_S640 = np.sqrt(640.0)                     # 640 = dxf*1e-5*1000*128^2*500
_SO = np.sqrt(640.0 / 2.75)                # Mo carries 1/(UO*BO) = 1/2.75
SIGW, BETW = 1.25 * _S640, -0.125 * _S640
SIGO, BETO = -1.25 * _SO, 1.125 * _SO
GSCALE = 0.25                              # k1/k2 ratio: 160/640


def _shift_matrices():
    """lhsT (=M^T) matrices for out = M @ p along the partition (x) axis."""
    sx = np.zeros((NX, NX), np.float32)    # f - b, edge clamped
    for i in range(NX):
        f, b = min(i + 1, NX - 1), max(i - 1, 0)
        sx[i, f] += 1.0
        sx[i, b] -= 1.0
    sxx = np.zeros((NX, NX), np.float32)   # f - 2c + b, edge clamped
    for i in range(NX):
        f, b = min(i + 1, NX - 1), max(i - 1, 0)
        sxx[i, f] += 1.0
        sxx[i, b] += 1.0
        sxx[i, i] -= 2.0
    m1 = sxx - 2.0 * np.eye(NX, dtype=np.float32)  # folds the y-center -2c
    ident = np.eye(NX, dtype=np.float32)
    return (np.ascontiguousarray(sx.T), np.ascontiguousarray(m1.T),
            ident, np.ascontiguousarray(-ident))


_NC_CACHE = {}


def _build_nc():
    import sys
    if '/opt/trn_rl_repo' not in sys.path:
        sys.path.insert(0, '/opt/trn_rl_repo')
    import concourse.bacc as bacc
    import concourse.tile as tile
    import concourse.mybir as mybir

    if 'nc' in _NC_CACHE:
        return _NC_CACHE['nc']

    CDT = mybir.dt.float16
    F32 = mybir.dt.float32
    AO = mybir.AluOpType
    AF = mybir.ActivationFunctionType

    nc = bacc.Bacc("TRN2", target_bir_lowering=False, debug=False,
                   enable_asserts=False, num_devices=N_CORES)

    press = nc.dram_tensor('press', [NX, T, NZ, NY], CDT, kind="ExternalInput").ap()
    perm = nc.dram_tensor('perm', [NX, T, NZ, NY], CDT, kind="ExternalInput").ap()
    q_in = nc.dram_tensor('q', [NX, T, NZ, NY], CDT, kind="ExternalInput").ap()
    qw_in = nc.dram_tensor('qw', [NX, T, NZ, NY], CDT, kind="ExternalInput").ap()
    sat_in = nc.dram_tensor('sat', [NX, T - 1, NZ, NY], CDT, kind="ExternalInput").ap()
    mw0_in = nc.dram_tensor('mw0', [NX, 1], F32, kind="ExternalInput").ap()
    mo0_in = nc.dram_tensor('mo0', [NX, 1], F32, kind="ExternalInput").ap()
    betw_in = nc.dram_tensor('betw', [NX, 1], F32, kind="ExternalInput").ap()
    beto_in = nc.dram_tensor('beto', [NX, 1], F32, kind="ExternalInput").ap()
    wsx_in = nc.dram_tensor('wsx', [NX, NX], CDT, kind="ExternalInput").ap()
    wm1_in = nc.dram_tensor('wm1', [NX, NX], CDT, kind="ExternalInput").ap()
    wid_in = nc.dram_tensor('wid', [NX, NX], CDT, kind="ExternalInput").ap()
    wni_in = nc.dram_tensor('wni', [NX, NX], CDT, kind="ExternalInput").ap()
    out_p = nc.dram_tensor('out_p', [NX, T, NZ, NY], CDT, kind="ExternalOutput").ap()
    out_s = nc.dram_tensor('out_s', [NX, T, NZ, NY], CDT, kind="ExternalOutput").ap()

    PW = NY + 4  # padded y width; data at [2:130], pads at 1 and 130

    with tile.TileContext(nc) as tc:
        with (
            tc.tile_pool(name="consts", bufs=1) as cpool,
            tc.tile_pool(name="stream", bufs=3) as spool,
            tc.tile_pool(name="inter", bufs=2) as ipool,
            tc.tile_pool(name="psum", bufs=2, space="PSUM") as ppool,
        ):
            wsx = cpool.tile([NX, NX], CDT, tag='wsx')
            wm1 = cpool.tile([NX, NX], CDT, tag='wm1')
            wid = cpool.tile([NX, NX], CDT, tag='wid')
            wni = cpool.tile([NX, NX], CDT, tag='wni')
            nc.sync.dma_start(wsx[:], wsx_in)
            nc.sync.dma_start(wm1[:], wm1_in)
            nc.sync.dma_start(wid[:], wid_in)
            nc.sync.dma_start(wni[:], wni_in)
            mw0 = cpool.tile([NX, 1], F32, tag='mw0')
            mo0 = cpool.tile([NX, 1], F32, tag='mo0')
            nc.sync.dma_start(mw0[:], mw0_in)
            nc.sync.dma_start(mo0[:], mo0_in)
            betw = cpool.tile([NX, 1], F32, tag='betw')
            beto = cpool.tile([NX, 1], F32, tag='beto')
            nc.sync.dma_start(betw[:], betw_in)
            nc.sync.dma_start(beto[:], beto_in)

            gw_x = cpool.tile([NX, NZ, NY], CDT, tag='gw_x')
            gw_y = cpool.tile([NX, NZ, NY], CDT, tag='gw_y')
            go_x = cpool.tile([NX, NZ, NY], CDT, tag='go_x')
            go_y = cpool.tile([NX, NZ, NY], CDT, tag='go_y')
            pa1w = cpool.tile([NX, NZ, PW], CDT, tag='pa1w')
            pa1o = cpool.tile([NX, NZ, PW], CDT, tag='pa1o')

            for t in range(T):
                # ---- streamed loads ----
                pbuf = spool.tile([NX, NZ, PW], CDT, tag='pbuf')
                nc.sync.dma_start(pbuf[:, :, 2:2 + NY], press[:, t])
                nc.scalar.copy(pbuf[:, :, 1:2], pbuf[:, :, 2:3])
                nc.scalar.copy(pbuf[:, :, 2 + NY:3 + NY], pbuf[:, :, 1 + NY:2 + NY])
                perm_t = spool.tile([NX, NZ, NY], CDT, tag='perm_t')
                nc.sync.dma_start(perm_t[:], perm[:, t])
                q_t = spool.tile([NX, NZ, NY], CDT, tag='q_t')
                nc.sync.dma_start(q_t[:], q_in[:, t])
                qw_t = spool.tile([NX, NZ, NY], CDT, tag='qw_t')
                nc.sync.dma_start(qw_t[:], qw_in[:, t])

                # ---- mobility fields ----
                if t == 0:
                    # prior == siniuse scalar -> per-partition scalar products
                    nc.scalar.activation(pa1w[:, :, 2:2 + NY], perm_t[:], AF.Copy,
                                         scale=mw0[:])
                    nc.scalar.activation(pa1o[:, :, 2:2 + NY], perm_t[:], AF.Copy,
                                         scale=mo0[:])
                    for pa in (pa1w, pa1o):
                        nc.scalar.copy(pa[:, :, 1:2], pa[:, :, 2:3])
                        nc.scalar.copy(pa[:, :, 2 + NY:3 + NY], pa[:, :, 1 + NY:2 + NY])
                    a1w = pa1w[:, :, 2:2 + NY]
                    a1o = pa1o[:, :, 2:2 + NY]
                else:
                    sat_t = spool.tile([NX, NZ, NY], CDT, tag='sat_t')
                    nc.sync.dma_start(sat_t[:], sat_in[:, t - 1])
                    mw2 = ipool.tile([NX, NZ, NY], CDT, tag='mw2')
                    mo2 = ipool.tile([NX, NZ, NY], CDT, tag='mo2')
                    nc.scalar.activation(mw2[:], sat_t[:], AF.Square,
                                         bias=betw[:], scale=SIGW)
                    nc.scalar.activation(mo2[:], sat_t[:], AF.Square,
                                         bias=beto[:], scale=SIGO)
                    a1w_t = ipool.tile([NX, NZ, NY], CDT, tag='a1w_t')
                    a1o_t = ipool.tile([NX, NZ, NY], CDT, tag='a1o_t')
                    nc.vector.tensor_mul(a1w_t[:], mw2[:], perm_t[:])
                    nc.vector.tensor_mul(a1o_t[:], mo2[:], perm_t[:])
                    a1w, a1o = a1w_t[:], a1o_t[:]

                # ---- pressure stencils on TensorE ----
                center = pbuf[:, :, 2:2 + NY]
                minus = pbuf[:, :, 1:1 + NY]
                plus = pbuf[:, :, 3:3 + NY]
                dx_ps = ppool.tile([NX, NZ, NY], F32, tag='dx')
                dy_ps = ppool.tile([NX, NZ, NY], F32, tag='dy')
                dd_ps = ppool.tile([NX, NZ, NY], F32, tag='dd')
                nc.tensor.matmul(dx_ps[:], wsx[:], center, start=True, stop=True)
                nc.tensor.matmul(dy_ps[:], wid[:], plus, start=True, stop=False)
                nc.tensor.matmul(dy_ps[:], wni[:], minus, start=False, stop=True)
                nc.tensor.matmul(dd_ps[:], wm1[:], center, start=True, stop=False)
                nc.tensor.matmul(dd_ps[:], wid[:], plus, start=False, stop=False)
                nc.tensor.matmul(dd_ps[:], wid[:], minus, start=False, stop=True)

                dxs = ipool.tile([NX, NZ, NY], CDT, tag='dxs')
                dys = ipool.tile([NX, NZ, NY], CDT, tag='dys')
                dds = ipool.tile([NX, NZ, NY], CDT, tag='dds')
                nc.scalar.copy(dxs[:], dx_ps[:])
                nc.scalar.copy(dys[:], dy_ps[:])
                nc.scalar.copy(dds[:], dd_ps[:])

                # ---- t=0: mobility-gradient fields (reuse dx/dy psum slots) ----
                if t == 0:
                    for pa, gx, gy in ((pa1w, gw_x, gw_y), (pa1o, go_x, go_y)):
                        gx_ps = ppool.tile([NX, NZ, NY], F32, tag='dx')
                        nc.tensor.matmul(gx_ps[:], wsx[:], pa[:, :, 2:2 + NY],
                                         start=True, stop=True)
                        nc.scalar.activation(gx[:], gx_ps[:], AF.Copy, scale=GSCALE)
                        gy_ps = ppool.tile([NX, NZ, NY], F32, tag='dy')
                        nc.tensor.matmul(gy_ps[:], wid[:], pa[:, :, 3:3 + NY],
                                         start=True, stop=False)
                        nc.tensor.matmul(gy_ps[:], wni[:], pa[:, :, 1:1 + NY],
                                         start=False, stop=True)
                        nc.scalar.activation(gy[:], gy_ps[:], AF.Copy, scale=GSCALE)

                # ---- loss assembly on VectorE ----
                mwx = ipool.tile([NX, NZ, NY], CDT, tag='mwx')
                mwy = ipool.tile([NX, NZ, NY], CDT, tag='mwy')
                mwd = ipool.tile([NX, NZ, NY], CDT, tag='mwd')
                nc.vector.tensor_mul(mwx[:], gw_x[:], dxs[:])
                nc.vector.tensor_mul(mwy[:], gw_y[:], dys[:])
                nc.vector.tensor_mul(mwd[:], a1w, dds[:])
                sw1 = ipool.tile([NX, NZ, NY], CDT, tag='sw1')
                sw = ipool.tile([NX, NZ, NY], CDT, tag='sw')
                nc.vector.tensor_add(sw1[:], mwx[:], mwy[:])
                nc.vector.tensor_add(sw[:], sw1[:], mwd[:])
                s_out = ipool.tile([NX, NZ, NY], CDT, tag='s_out')
                nc.vector.scalar_tensor_tensor(s_out[:], qw_t[:], -CQ, sw[:],
                                               op0=AO.mult, op1=AO.subtract)

                mox = ipool.tile([NX, NZ, NY], CDT, tag='mox')
                moy = ipool.tile([NX, NZ, NY], CDT, tag='moy')
                mod = ipool.tile([NX, NZ, NY], CDT, tag='mod')
                nc.vector.tensor_mul(mox[:], go_x[:], dxs[:])
                nc.vector.tensor_mul(moy[:], go_y[:], dys[:])
                nc.vector.tensor_mul(mod[:], a1o, dds[:])
                so1 = ipool.tile([NX, NZ, NY], CDT, tag='so1')
                so = ipool.tile([NX, NZ, NY], CDT, tag='so')
                nc.vector.tensor_add(so1[:], mox[:], moy[:])
                nc.vector.tensor_add(so[:], so1[:], mod[:])
                p1 = ipool.tile([NX, NZ, NY], CDT, tag='p1')
                nc.vector.scalar_tensor_tensor(p1[:], q_t[:], CQ, so[:],
                                               op0=AO.mult, op1=AO.add)
                p_out = ipool.tile([NX, NZ, NY], CDT, tag='p_out')
                nc.vector.tensor_add(p_out[:], p1[:], sw[:])

                nc.sync.dma_start(out_p[:, t], p_out[:])
                nc.sync.dma_start(out_s[:, t], s_out[:])

    nc.compile()
    _NC_CACHE['nc'] = nc
    return nc


def kernel(pressure, perm, Q, Qw, Time, Pini, Phi, Swini, water_sat):
    import sys
    if '/opt/trn_rl_repo' not in sys.path:
        sys.path.insert(0, '/opt/trn_rl_repo')
    from concourse.bass_utils import run_bass_kernel_spmd

    nc = _build_nc()

    sini = float(np.asarray(Swini[0, 0, 0, 0, 0]))
    mw0 = np.float32((SIGW * sini + BETW) ** 2)
    mo0 = np.float32((SIGO * sini + BETO) ** 2)
    mw0_col = np.full((NX, 1), mw0, np.float32)
    mo0_col = np.full((NX, 1), mo0, np.float32)
    sxT, m1T, idm, nim = _shift_matrices()
    consts = {
        'mw0': mw0_col, 'mo0': mo0_col,
        'betw': np.full((NX, 1), BETW, np.float32),
        'beto': np.full((NX, 1), BETO, np.float32),
        'wsx': sxT.astype(np.float16), 'wm1': m1T.astype(np.float16),
        'wid': idm.astype(np.float16), 'wni': nim.astype(np.float16),
    }

    def to_xtzy(a):  # [T,NZ,NX,NY] -> [NX,T,NZ,NY], fp16, contiguous
        return np.ascontiguousarray(np.asarray(a).transpose(2, 0, 1, 3),
                                    dtype=np.float16)

    in_maps = []
    for c in range(N_CORES):
        in_maps.append({
            'press': to_xtzy(pressure[c]),
            'perm': to_xtzy(perm[c]),
            'q': to_xtzy(Q[c]),
            'qw': to_xtzy(Qw[c]),
            'sat': to_xtzy(water_sat[c, :T - 1]),
            **consts,
        })

    res = run_bass_kernel_spmd(nc, in_maps, core_ids=list(range(N_CORES)))

    p_loss = np.empty((B, T, NZ, NX, NY), np.float32)
    s_loss = np.empty((B, T, NZ, NX, NY), np.float32)
    for c in range(N_CORES):
        p_loss[c] = res.results[c]['out_p'].astype(np.float32).transpose(1, 2, 0, 3)
        s_loss[c] = res.results[c]['out_s'].astype(np.float32).transpose(1, 2, 0, 3)
    return p_loss, s_loss


# revision 6
# speedup vs baseline: 1.0051x; 1.0051x over previous
"""Black-oil PINO loss kernel for 8 Trainium2 NeuronCores.

Contract: kernel(**inputs) takes FULL f32 inputs [B=8,T=10,NZ=4,NX=128,NY=128]
and returns (p_loss, s_loss) as full f32 arrays, computed on 8 NeuronCores
(batch sharded, one batch element per core, no cross-core communication).

Math (constant-folded from the reference):
    prior   = shift_t(water_sat), prior[0] = Swini[0,0,0,0,0]
    Mw''    = Square(sigw*prior + betw)          # = 640*Mw
    Mo''    = Square(sigo*prior + beto)          # = 640*Mo
    a1w''   = Mw''*perm ; a1o'' = Mo''*perm      # = 640*Mw*a, 640*Mo*a
    Dx/Dy   = raw central diff (f-b), DD = f-2c+b, edge-replicated
    GW_*    = 0.25*D*(a1w''[t=0]) ; GO_* = 0.25*D*(a1o''[t=0])
    dd      = DDx(p) + DDy(p)
    sw      = GW_x*Dx(p) + GW_y*Dy(p) + a1w''*dd
    so      = GO_x*Dx(p) + GO_y*Dy(p) + a1o''*dd
    p_loss  = cQ*Q + sw + so
    s_loss  = -(cQ*Qw + sw)
The saturation-accumulation term Phi*(dsw/dta)*dxf*1e-5 is <= 2.4e-10 while
|s_loss| ~ 2.7e3 (13 orders below f32 output noise), so it is dropped; Phi,
Time, Pini are then unused and never shipped to the device.

Device-side layout is [x(partitions), t, z, y(contiguous)], fp16. The host
pre-pads pressure along y (edge-replicated, width 132) and folds the cQ
scale into the fp16 cast of Q/Qw. x stencils run on TensorE as 128x128
shift-matrix matmuls; y stencils are Id/-Id matmuls over y-shifted access
patterns of the padded pressure, accumulated into the same PSUM banks.
ScalarE does the Square mobilities and all PSUM->SBUF fp16 copies; VectorE
does the remaining elementwise products/sums in 2-timestep blocks.
"""

import numpy as np

B, T, NZ, NX, NY = 8, 10, 4, 128, 128
N_CORES = 8
TB = 2            # timesteps per elementwise block
PW = NY + 4       # padded y width; data at [2:130], edge pads at 1 and 130

# folded constants
CQ = 5000.0 * 1e-5 / 128.0                 # dxf*1e-5*UIR
_S640 = np.sqrt(640.0)                     # 640 = dxf*1e-5*1000*128^2*500
_SO = np.sqrt(640.0 / 2.75)                # Mo carries 1/(UO*BO) = 1/2.75
SIGW, BETW = 1.25 * _S640, -0.125 * _S640
SIGO, BETO = -1.25 * _SO, 1.125 * _SO
GSCALE = 0.25                              # k1/k2 ratio: 160/640


def _shift_matrices():
    """lhsT (=M^T) matrices for out = M @ p along the partition (x) axis."""
    sx = np.zeros((NX, NX), np.float32)    # f - b, edge clamped
    for i in range(NX):
        f, b = min(i + 1, NX - 1), max(i - 1, 0)
        sx[i, f] += 1.0
        sx[i, b] -= 1.0
    sxx = np.zeros((NX, NX), np.float32)   # f - 2c + b, edge clamped
    for i in range(NX):
        f, b = min(i + 1, NX - 1), max(i - 1, 0)
        sxx[i, f] += 1.0
        sxx[i, b] += 1.0
        sxx[i, i] -= 2.0
    m1 = sxx - 2.0 * np.eye(NX, dtype=np.float32)  # folds the y-center -2c
    ident = np.eye(NX, dtype=np.float32)
    return (np.ascontiguousarray(sx.T), np.ascontiguousarray(m1.T),
            ident, np.ascontiguousarray(-ident))


_NC_CACHE = {}


def _build_nc():
    import sys
    if '/opt/trn_rl_repo' not in sys.path:
        sys.path.insert(0, '/opt/trn_rl_repo')
    import concourse.bacc as bacc
    import concourse.tile as tile
    import concourse.mybir as mybir

    if 'nc' in _NC_CACHE:
        return _NC_CACHE['nc']

    CDT = mybir.dt.float16
    F32 = mybir.dt.float32
    AF = mybir.ActivationFunctionType

    nc = bacc.Bacc("TRN2", target_bir_lowering=False, debug=False,
                   enable_asserts=False, num_devices=N_CORES)

    press = nc.dram_tensor('press', [NX, T, NZ, PW], CDT, kind="ExternalInput").ap()
    perm = nc.dram_tensor('perm', [NX, T, NZ, NY], CDT, kind="ExternalInput").ap()
    qs_in = nc.dram_tensor('qs', [NX, T, NZ, NY], CDT, kind="ExternalInput").ap()
    qws_in = nc.dram_tensor('qws', [NX, T, NZ, NY], CDT, kind="ExternalInput").ap()
    sat_in = nc.dram_tensor('sat', [NX, T - 1, NZ, NY], CDT, kind="ExternalInput").ap()
    mw0_in = nc.dram_tensor('mw0', [NX, 1], F32, kind="ExternalInput").ap()
    mo0_in = nc.dram_tensor('mo0', [NX, 1], F32, kind="ExternalInput").ap()
    betw_in = nc.dram_tensor('betw', [NX, 1], F32, kind="ExternalInput").ap()
    beto_in = nc.dram_tensor('beto', [NX, 1], F32, kind="ExternalInput").ap()
    wsx_in = nc.dram_tensor('wsx', [NX, NX], CDT, kind="ExternalInput").ap()
    wm1_in = nc.dram_tensor('wm1', [NX, NX], CDT, kind="ExternalInput").ap()
    wid_in = nc.dram_tensor('wid', [NX, NX], CDT, kind="ExternalInput").ap()
    wni_in = nc.dram_tensor('wni', [NX, NX], CDT, kind="ExternalInput").ap()
    out_p = nc.dram_tensor('out_p', [NX, T, NZ, NY], CDT, kind="ExternalOutput").ap()
    out_s = nc.dram_tensor('out_s', [NX, T, NZ, NY], CDT, kind="ExternalOutput").ap()

    NBLK = T // TB

    with tile.TileContext(nc) as tc:
        with (
            tc.tile_pool(name="consts", bufs=1) as cpool,
            tc.tile_pool(name="big", bufs=1) as bpool,
            tc.tile_pool(name="stage", bufs=3) as gpool,
            tc.tile_pool(name="work", bufs=2) as wpool,
            tc.tile_pool(name="psum", bufs=2, space="PSUM") as ppool,
        ):
            # ---- constants ----
            wsx = cpool.tile([NX, NX], CDT, tag='wsx')
            wm1 = cpool.tile([NX, NX], CDT, tag='wm1')
            wid = cpool.tile([NX, NX], CDT, tag='wid')
            wni = cpool.tile([NX, NX], CDT, tag='wni')
            nc.sync.dma_start(wsx[:], wsx_in)
            nc.sync.dma_start(wm1[:], wm1_in)
            nc.sync.dma_start(wid[:], wid_in)
            nc.sync.dma_start(wni[:], wni_in)
            mw0 = cpool.tile([NX, 1], F32, tag='mw0')
            mo0 = cpool.tile([NX, 1], F32, tag='mo0')
            betw = cpool.tile([NX, 1], F32, tag='betw')
            beto = cpool.tile([NX, 1], F32, tag='beto')
            nc.sync.dma_start(mw0[:], mw0_in)
            nc.sync.dma_start(mo0[:], mo0_in)
            nc.sync.dma_start(betw[:], betw_in)
            nc.sync.dma_start(beto[:], beto_in)

            # ---- whole-tensor input loads ----
            sat_all = bpool.tile([NX, T - 1, NZ, NY], CDT, tag='sat_all')
            nc.sync.dma_start(sat_all[:], sat_in)
            perm_all = bpool.tile([NX, T, NZ, NY], CDT, tag='perm_all')
            nc.sync.dma_start(perm_all[:], perm)
            press_all = bpool.tile([NX, T, NZ, PW], CDT, tag='press_all')
            nc.sync.dma_start(press_all[:], press)
            qs_all = bpool.tile([NX, T, NZ, NY], CDT, tag='qs_all')
            nc.sync.dma_start(qs_all[:], qs_in)
            qws_all = bpool.tile([NX, T, NZ, NY], CDT, tag='qws_all')
            nc.sync.dma_start(qws_all[:], qws_in)

            # ---- mobilities for t>=1 (batched) ----
            mw2 = bpool.tile([NX, T - 1, NZ, NY], CDT, tag='mw2')
            mo2 = bpool.tile([NX, T - 1, NZ, NY], CDT, tag='mo2')
            nc.scalar.activation(mw2[:], sat_all[:], AF.Square, bias=betw[:], scale=SIGW)
            nc.scalar.activation(mo2[:], sat_all[:], AF.Square, bias=beto[:], scale=SIGO)
            a1w_all = bpool.tile([NX, T, NZ, NY], CDT, tag='a1w_all')
            a1o_all = bpool.tile([NX, T, NZ, NY], CDT, tag='a1o_all')
            # t=0 slab: prior is the Swini scalar -> per-partition scalar products
            nc.scalar.activation(a1w_all[:, 0], perm_all[:, 0], AF.Copy, scale=mw0[:])
            nc.scalar.activation(a1o_all[:, 0], perm_all[:, 0], AF.Copy, scale=mo0[:])
            nc.vector.tensor_mul(a1w_all[:, 1:T], mw2[:], perm_all[:, 1:T])
            nc.vector.tensor_mul(a1o_all[:, 1:T], mo2[:], perm_all[:, 1:T])

            # ---- t=0 mobility-gradient fields ----
            gw_x = cpool.tile([NX, NZ, NY], CDT, tag='gw_x')
            gw_y = cpool.tile([NX, NZ, NY], CDT, tag='gw_y')
            go_x = cpool.tile([NX, NZ, NY], CDT, tag='go_x')
            go_y = cpool.tile([NX, NZ, NY], CDT, tag='go_y')
            pa1w = cpool.tile([NX, NZ, PW], CDT, tag='pa1w')
            pa1o = cpool.tile([NX, NZ, PW], CDT, tag='pa1o')
            for pa, a_all in ((pa1w, a1w_all), (pa1o, a1o_all)):
                nc.scalar.copy(pa[:, :, 2:2 + NY], a_all[:, 0])
                nc.scalar.copy(pa[:, :, 1:2], a_all[:, 0, :, 0:1])
                nc.scalar.copy(pa[:, :, 2 + NY:3 + NY], a_all[:, 0, :, NY - 1:NY])
            for pa, a_all, gx, gy in ((pa1w, a1w_all, gw_x, gw_y),
                                      (pa1o, a1o_all, go_x, go_y)):
                gx_ps = ppool.tile([NX, NZ, NY], F32, tag='dx')
                nc.tensor.matmul(gx_ps[:], wsx[:], a_all[:, 0], start=True, stop=True)
                nc.scalar.activation(gx[:], gx_ps[:], AF.Copy, scale=GSCALE)
                gy_ps = ppool.tile([NX, NZ, NY], F32, tag='dy')
                nc.tensor.matmul(gy_ps[:], wid[:], pa[:, :, 3:3 + NY],
                                 start=True, stop=False)
                nc.tensor.matmul(gy_ps[:], wni[:], pa[:, :, 1:1 + NY],
                                 start=False, stop=True)
                nc.scalar.activation(gy[:], gy_ps[:], AF.Copy, scale=GSCALE)

            bgw_x = gw_x[:].unsqueeze(1).to_broadcast((NX, TB, NZ, NY))
            bgw_y = gw_y[:].unsqueeze(1).to_broadcast((NX, TB, NZ, NY))
            bgo_x = go_x[:].unsqueeze(1).to_broadcast((NX, TB, NZ, NY))
            bgo_y = go_y[:].unsqueeze(1).to_broadcast((NX, TB, NZ, NY))

            # ---- per-block: derivatives on PE, losses on DVE ----
            for b in range(NBLK):
                t0 = b * TB
                dxs = gpool.tile([NX, TB, NZ, NY], CDT, tag='dxs')
                dys = gpool.tile([NX, TB, NZ, NY], CDT, tag='dys')
                dds = gpool.tile([NX, TB, NZ, NY], CDT, tag='dds')
                for i in range(TB):
                    t = t0 + i
                    center = press_all[:, t, :, 2:2 + NY]
                    minus = press_all[:, t, :, 1:1 + NY]
                    plus = press_all[:, t, :, 3:3 + NY]
                    dx_ps = ppool.tile([NX, NZ, NY], F32, tag='dx')
                    dy_ps = ppool.tile([NX, NZ, NY], F32, tag='dy')
                    dd_ps = ppool.tile([NX, NZ, NY], F32, tag='dd')
                    nc.tensor.matmul(dx_ps[:], wsx[:], center, start=True, stop=True)
                    nc.tensor.matmul(dy_ps[:], wid[:], plus, start=True, stop=False)
                    nc.tensor.matmul(dy_ps[:], wni[:], minus, start=False, stop=True)
                    nc.tensor.matmul(dd_ps[:], wm1[:], center, start=True, stop=False)
                    nc.tensor.matmul(dd_ps[:], wid[:], plus, start=False, stop=False)
                    nc.tensor.matmul(dd_ps[:], wid[:], minus, start=False, stop=True)
                    nc.scalar.copy(dxs[:, i], dx_ps[:])
                    nc.scalar.copy(dys[:, i], dy_ps[:])
                    nc.scalar.copy(dds[:, i], dd_ps[:])

                tsl = slice(t0, t0 + TB)
                mwx = wpool.tile([NX, TB, NZ, NY], CDT, tag='mwx')
                mwy = wpool.tile([NX, TB, NZ, NY], CDT, tag='mwy')
                mwd = wpool.tile([NX, TB, NZ, NY], CDT, tag='mwd')
                nc.vector.tensor_mul(mwx[:], bgw_x, dxs[:])
                nc.vector.tensor_mul(mwy[:], bgw_y, dys[:])
                nc.vector.tensor_mul(mwd[:], a1w_all[:, tsl], dds[:])
                sw1 = wpool.tile([NX, TB, NZ, NY], CDT, tag='sw1')
                sw = wpool.tile([NX, TB, NZ, NY], CDT, tag='sw')
                nc.vector.tensor_add(sw1[:], mwx[:], mwy[:])
                nc.vector.tensor_add(sw[:], sw1[:], mwd[:])
                s_out = wpool.tile([NX, TB, NZ, NY], CDT, tag='s_out')
                nc.vector.tensor_sub(s_out[:], qws_all[:, tsl], sw[:])
                nc.sync.dma_start(out_s[:, tsl], s_out[:])

                mox = wpool.tile([NX, TB, NZ, NY], CDT, tag='mox')
                moy = wpool.tile([NX, TB, NZ, NY], CDT, tag='moy')
                mod = wpool.tile([NX, TB, NZ, NY], CDT, tag='mod')
                nc.vector.tensor_mul(mox[:], bgo_x, dxs[:])
                nc.vector.tensor_mul(moy[:], bgo_y, dys[:])
                nc.vector.tensor_mul(mod[:], a1o_all[:, tsl], dds[:])
                so1 = wpool.tile([NX, TB, NZ, NY], CDT, tag='so1')
                so = wpool.tile([NX, TB, NZ, NY], CDT, tag='so')
                nc.vector.tensor_add(so1[:], mox[:], moy[:])
                nc.vector.tensor_add(so[:], so1[:], mod[:])
                p1 = wpool.tile([NX, TB, NZ, NY], CDT, tag='p1')
                p_out = wpool.tile([NX, TB, NZ, NY], CDT, tag='p_out')
                nc.vector.tensor_add(p1[:], qs_all[:, tsl], so[:])
                nc.vector.tensor_add(p_out[:], p1[:], sw[:])
                nc.sync.dma_start(out_p[:, tsl], p_out[:])

    nc.compile()
    _NC_CACHE['nc'] = nc
    return nc


def kernel(pressure, perm, Q, Qw, Time, Pini, Phi, Swini, water_sat):
    import sys
    if '/opt/trn_rl_repo' not in sys.path:
        sys.path.insert(0, '/opt/trn_rl_repo')
    from concourse.bass_utils import run_bass_kernel_spmd

    nc = _build_nc()

    sini = float(np.asarray(Swini[0, 0, 0, 0, 0]))
    mw0 = np.float32((SIGW * sini + BETW) ** 2)
    mo0 = np.float32((SIGO * sini + BETO) ** 2)
    sxT, m1T, idm, nim = _shift_matrices()
    consts = {
        'mw0': np.full((NX, 1), mw0, np.float32),
        'mo0': np.full((NX, 1), mo0, np.float32),
        'betw': np.full((NX, 1), BETW, np.float32),
        'beto': np.full((NX, 1), BETO, np.float32),
        'wsx': sxT.astype(np.float16), 'wm1': m1T.astype(np.float16),
        'wid': idm.astype(np.float16), 'wni': nim.astype(np.float16),
    }

    def to_xtzy(a, scale=None):  # [T,NZ,NX,NY] -> [NX,T,NZ,NY] fp16 contiguous
        a = np.asarray(a).transpose(2, 0, 1, 3)
        if scale is not None:
            a = a * scale
        return np.ascontiguousarray(a, dtype=np.float16)

    def pad_press(a):  # [T,NZ,NX,NY] -> [NX,T,NZ,NY+4] edge-padded fp16
        x = np.asarray(a).transpose(2, 0, 1, 3)
        out = np.zeros((NX, T, NZ, PW), np.float16)
        out[..., 2:2 + NY] = x
        out[..., 1] = x[..., 0]
        out[..., 2 + NY] = x[..., NY - 1]
        return out

    in_maps = []
    for c in range(N_CORES):
        in_maps.append({
            'press': pad_press(pressure[c]),
            'perm': to_xtzy(perm[c]),
            'qs': to_xtzy(Q[c], CQ),
            'qws': to_xtzy(Qw[c], -CQ),
            'sat': to_xtzy(water_sat[c, :T - 1]),
            **consts,
        })

    res = run_bass_kernel_spmd(nc, in_maps, core_ids=list(range(N_CORES)))

    p_loss = np.empty((B, T, NZ, NX, NY), np.float32)
    s_loss = np.empty((B, T, NZ, NX, NY), np.float32)
    for c in range(N_CORES):
        p_loss[c] = res.results[c]['out_p'].astype(np.float32).transpose(1, 2, 0, 3)
        s_loss[c] = res.results[c]['out_s'].astype(np.float32).transpose(1, 2, 0, 3)
    return p_loss, s_loss


# revision 8
# speedup vs baseline: 1.1803x; 1.1742x over previous
"""Black-oil PINO loss kernel for 8 Trainium2 NeuronCores.

Contract: kernel(**inputs) takes FULL f32 inputs [B=8,T=10,NZ=4,NX=128,NY=128]
and returns (p_loss, s_loss) as full f32 arrays, computed on 8 NeuronCores
(batch sharded, one batch element per core, no cross-core communication).

Math (constant-folded from the reference):
    prior    = shift_t(water_sat), prior[0] = siniuse = Swini[0,0,0,0,0]
    mw2      = Square(sigw*prior + betw)         # = 640*Mw
    mo2      = Square(sigo*prior + beto)         # = 640*Mo
    Dx/Dy    = raw central diff (f-b), DD = f-2c+b, edge-replicated
    dd       = DDx(p) + DDy(p)
    pd       = perm*dd
    U        = Dx(perm0)*Dx(p) + Dy(perm0)*Dy(p)
    sw       = cw*U + mw2*pd        # cw = 0.25*mw2(siniuse): t=0 prior is a
    so       = co*U + mo2*pd        # scalar, so grad(a1_0) = c * grad(perm0)
    p_loss   = cQ*Q + sw + so
    s_loss   = -(cQ*Qw + sw)
The saturation-accumulation term Phi*(dsw/dta)*dxf*1e-5 is <= 2.4e-10 while
|s_loss| ~ 2.7e3 (13 orders below f32 output noise), so it is dropped; Phi,
Time, Pini are then unused and never shipped to the device.

Device-side layout is [x(partitions), t, z, y(contiguous)], fp16. The host
pre-pads pressure/perm0 along y (edge replication) and folds the cQ scale
into the fp16 cast of Q/Qw. x stencils run on TensorE as 128x128
shift-matrix matmuls; y stencils are Id/-Id matmuls over y-shifted access
patterns of the padded pressure, accumulated into PSUM. Dx/Dy/dd land in
one 3-bank PSUM tile per timestep, moved to SBUF fp16 by a single ScalarE
copy. ScalarE also computes the Square mobilities; VectorE runs the 13
remaining elementwise ops per 2-timestep block.
"""

import numpy as np

B, T, NZ, NX, NY = 8, 10, 4, 128, 128
N_CORES = 8
TB = 2            # timesteps per elementwise block
NBLK = T // TB
PW = NY + 4       # padded y width; data at [2:130], edge pads at 1 and 130

# folded constants
CQ = 5000.0 * 1e-5 / 128.0                 # dxf*1e-5*UIR
_S640 = np.sqrt(640.0)                     # 640 = dxf*1e-5*1000*128^2*500
_SO = np.sqrt(640.0 / 2.75)                # Mo carries 1/(UO*BO) = 1/2.75
SIGW, BETW = 1.25 * _S640, -0.125 * _S640
SIGO, BETO = -1.25 * _SO, 1.125 * _SO
GSCALE = 0.25                              # k1/k2 ratio: 160/640


def _shift_matrices():
    """lhsT (=M^T) matrices for out = M @ p along the partition (x) axis."""
    sx = np.zeros((NX, NX), np.float32)    # f - b, edge clamped
    for i in range(NX):
        f, b = min(i + 1, NX - 1), max(i - 1, 0)
        sx[i, f] += 1.0
        sx[i, b] -= 1.0
    sxx = np.zeros((NX, NX), np.float32)   # f - 2c + b, edge clamped
    for i in range(NX):
        f, b = min(i + 1, NX - 1), max(i - 1, 0)
        sxx[i, f] += 1.0
        sxx[i, b] += 1.0
        sxx[i, i] -= 2.0
    m1 = sxx - 2.0 * np.eye(NX, dtype=np.float32)  # folds the y-center -2c
    ident = np.eye(NX, dtype=np.float32)
    return (np.ascontiguousarray(sx.T), np.ascontiguousarray(m1.T),
            ident, np.ascontiguousarray(-ident))


_NC_CACHE = {}


def _build_nc():
    import sys
    if '/opt/trn_rl_repo' not in sys.path:
        sys.path.insert(0, '/opt/trn_rl_repo')
    import concourse.bacc as bacc
    import concourse.tile as tile
    import concourse.mybir as mybir

    if 'nc' in _NC_CACHE:
        return _NC_CACHE['nc']

    CDT = mybir.dt.float16
    F32 = mybir.dt.float32
    AO = mybir.AluOpType
    AF = mybir.ActivationFunctionType

    nc = bacc.Bacc("TRN2", target_bir_lowering=False, debug=False,
                   enable_asserts=False, num_devices=N_CORES)

    press = nc.dram_tensor('press', [NX, T, NZ, PW], CDT, kind="ExternalInput").ap()
    perm = nc.dram_tensor('perm', [NX, T, NZ, NY], CDT, kind="ExternalInput").ap()
    perm0p_in = nc.dram_tensor('perm0p', [NX, NZ, PW], CDT, kind="ExternalInput").ap()
    qs_in = nc.dram_tensor('qs', [NX, T, NZ, NY], CDT, kind="ExternalInput").ap()
    qws_in = nc.dram_tensor('qws', [NX, T, NZ, NY], CDT, kind="ExternalInput").ap()
    sat_in = nc.dram_tensor('sat', [NX, T - 1, NZ, NY], CDT, kind="ExternalInput").ap()
    mw0_in = nc.dram_tensor('mw0', [NX, 1], F32, kind="ExternalInput").ap()
    mo0_in = nc.dram_tensor('mo0', [NX, 1], F32, kind="ExternalInput").ap()
    cw_in = nc.dram_tensor('cw', [NX, 1], F32, kind="ExternalInput").ap()
    co_in = nc.dram_tensor('co', [NX, 1], F32, kind="ExternalInput").ap()
    betw_in = nc.dram_tensor('betw', [NX, 1], F32, kind="ExternalInput").ap()
    beto_in = nc.dram_tensor('beto', [NX, 1], F32, kind="ExternalInput").ap()
    wsx_in = nc.dram_tensor('wsx', [NX, NX], CDT, kind="ExternalInput").ap()
    wm1_in = nc.dram_tensor('wm1', [NX, NX], CDT, kind="ExternalInput").ap()
    wid_in = nc.dram_tensor('wid', [NX, NX], CDT, kind="ExternalInput").ap()
    wni_in = nc.dram_tensor('wni', [NX, NX], CDT, kind="ExternalInput").ap()
    out_p = nc.dram_tensor('out_p', [NX, T, NZ, NY], CDT, kind="ExternalOutput").ap()
    out_s = nc.dram_tensor('out_s', [NX, T, NZ, NY], CDT, kind="ExternalOutput").ap()

    with tile.TileContext(nc) as tc:
        with (
            tc.tile_pool(name="consts", bufs=1) as cpool,
            tc.tile_pool(name="stream", bufs=3) as spool,
            tc.tile_pool(name="work", bufs=2) as wpool,
            tc.tile_pool(name="psum", bufs=2, space="PSUM") as ppool,
            tc.tile_pool(name="gsum", bufs=1, space="PSUM") as gppool,
        ):
            # ---- constants ----
            wsx = cpool.tile([NX, NX], CDT, tag='wsx')
            wm1 = cpool.tile([NX, NX], CDT, tag='wm1')
            wid = cpool.tile([NX, NX], CDT, tag='wid')
            wni = cpool.tile([NX, NX], CDT, tag='wni')
            nc.sync.dma_start(wsx[:], wsx_in)
            nc.sync.dma_start(wm1[:], wm1_in)
            nc.sync.dma_start(wid[:], wid_in)
            nc.sync.dma_start(wni[:], wni_in)
            cols = {}
            for nm, src in (('mw0', mw0_in), ('mo0', mo0_in), ('cw', cw_in),
                            ('co', co_in), ('betw', betw_in), ('beto', beto_in)):
                cols[nm] = cpool.tile([NX, 1], F32, tag=nm, name=nm)
                nc.sync.dma_start(cols[nm][:], src)
            perm0p = cpool.tile([NX, NZ, PW], CDT, tag='perm0p')
            nc.sync.dma_start(perm0p[:], perm0p_in)

            # ---- grad(perm0) setup fields ----
            dpx = cpool.tile([NX, NZ, NY], CDT, tag='dpx')
            dpy = cpool.tile([NX, NZ, NY], CDT, tag='dpy')
            c0 = perm0p[:, :, 2:2 + NY]
            dpx_ps = gppool.tile([NX, NZ, NY], F32, tag='gps')
            nc.tensor.matmul(dpx_ps[:], wsx[:], c0, start=True, stop=True)
            nc.scalar.copy(dpx[:], dpx_ps[:])
            dpy_ps = gppool.tile([NX, NZ, NY], F32, tag='gps')
            nc.tensor.matmul(dpy_ps[:], wid[:], perm0p[:, :, 3:3 + NY],
                             start=True, stop=False)
            nc.tensor.matmul(dpy_ps[:], wni[:], perm0p[:, :, 1:1 + NY],
                             start=False, stop=True)
            nc.scalar.copy(dpy[:], dpy_ps[:])
            bdpx = dpx[:].unsqueeze(1).to_broadcast((NX, TB, NZ, NY))
            bdpy = dpy[:].unsqueeze(1).to_broadcast((NX, TB, NZ, NY))

            # ---- per-block pipeline ----
            for b in range(NBLK):
                t0 = b * TB
                tsl = slice(t0, t0 + TB)
                pr_b = spool.tile([NX, TB, NZ, PW], CDT, tag='pr_b')
                nc.sync.dma_start(pr_b[:], press[:, tsl])
                pe_b = spool.tile([NX, TB, NZ, NY], CDT, tag='pe_b')
                nc.sync.dma_start(pe_b[:], perm[:, tsl])
                qs_b = spool.tile([NX, TB, NZ, NY], CDT, tag='qs_b')
                nc.sync.dma_start(qs_b[:], qs_in[:, tsl])
                qws_b = spool.tile([NX, TB, NZ, NY], CDT, tag='qws_b')
                nc.sync.dma_start(qws_b[:], qws_in[:, tsl])

                # mobility squares (prior = sat shifted by one t)
                mw2 = spool.tile([NX, TB, NZ, NY], CDT, tag='mw2')
                mo2 = spool.tile([NX, TB, NZ, NY], CDT, tag='mo2')
                if b == 0:
                    sat_b = spool.tile([NX, 1, NZ, NY], CDT, tag='sat_b')
                    nc.sync.dma_start(sat_b[:], sat_in[:, 0:1])
                    # t=0 prior is the siniuse scalar
                    nc.scalar.activation(mw2[:, 0], pe_b[:, 0], AF.Identity,
                                         bias=cols['mw0'][:], scale=0.0)
                    nc.scalar.activation(mo2[:, 0], pe_b[:, 0], AF.Identity,
                                         bias=cols['mo0'][:], scale=0.0)
                    nc.scalar.activation(mw2[:, 1], sat_b[:, 0], AF.Square,
                                         bias=cols['betw'][:], scale=SIGW)
                    nc.scalar.activation(mo2[:, 1], sat_b[:, 0], AF.Square,
                                         bias=cols['beto'][:], scale=SIGO)
                else:
                    sat_b = spool.tile([NX, TB, NZ, NY], CDT, tag='sat_b')
                    nc.sync.dma_start(sat_b[:], sat_in[:, t0 - 1:t0 - 1 + TB])
                    nc.scalar.activation(mw2[:], sat_b[:], AF.Square,
                                         bias=cols['betw'][:], scale=SIGW)
                    nc.scalar.activation(mo2[:], sat_b[:], AF.Square,
                                         bias=cols['beto'][:], scale=SIGO)

                # pressure stencils: Dx, Dy, DD into one 3-bank PSUM tile per t
                stg = spool.tile([NX, TB, 3, NZ, NY], CDT, tag='stg')
                for i in range(TB):
                    t = t0 + i
                    center = pr_b[:, i, :, 2:2 + NY]
                    minus = pr_b[:, i, :, 1:1 + NY]
                    plus = pr_b[:, i, :, 3:3 + NY]
                    ps = ppool.tile([NX, 3, NZ, NY], F32, tag='ps')
                    nc.tensor.matmul(ps[:, 0], wsx[:], center, start=True, stop=True)
                    nc.tensor.matmul(ps[:, 1], wid[:], plus, start=True, stop=False)
                    nc.tensor.matmul(ps[:, 1], wni[:], minus, start=False, stop=True)
                    nc.tensor.matmul(ps[:, 2], wm1[:], center, start=True, stop=False)
                    nc.tensor.matmul(ps[:, 2], wid[:], plus, start=False, stop=False)
                    nc.tensor.matmul(ps[:, 2], wid[:], minus, start=False, stop=True)
                    nc.scalar.copy(stg[:, i], ps[:])

                dxs = stg[:, :, 0]
                dys = stg[:, :, 1]
                dds = stg[:, :, 2]

                # elementwise assembly on VectorE
                pd = wpool.tile([NX, TB, NZ, NY], CDT, tag='pd')
                ux = wpool.tile([NX, TB, NZ, NY], CDT, tag='ux')
                uy = wpool.tile([NX, TB, NZ, NY], CDT, tag='uy')
                uu = wpool.tile([NX, TB, NZ, NY], CDT, tag='uu')
                nc.vector.tensor_mul(pd[:], pe_b[:], dds)
                nc.vector.tensor_mul(ux[:], bdpx, dxs)
                nc.vector.tensor_mul(uy[:], bdpy, dys)
                nc.vector.tensor_add(uu[:], ux[:], uy[:])
                mwd = wpool.tile([NX, TB, NZ, NY], CDT, tag='mwd')
                mod = wpool.tile([NX, TB, NZ, NY], CDT, tag='mod')
                nc.vector.tensor_mul(mwd[:], mw2[:], pd[:])
                nc.vector.tensor_mul(mod[:], mo2[:], pd[:])
                cwu = wpool.tile([NX, TB, NZ, NY], CDT, tag='cwu')
                cou = wpool.tile([NX, TB, NZ, NY], CDT, tag='cou')
                nc.vector.tensor_scalar(cwu[:], uu[:], cols['cw'][:], None, op0=AO.mult)
                nc.vector.tensor_scalar(cou[:], uu[:], cols['co'][:], None, op0=AO.mult)
                sw = wpool.tile([NX, TB, NZ, NY], CDT, tag='sw')
                so = wpool.tile([NX, TB, NZ, NY], CDT, tag='so')
                nc.vector.tensor_add(sw[:], cwu[:], mwd[:])
                nc.vector.tensor_add(so[:], cou[:], mod[:])
                s_out = wpool.tile([NX, TB, NZ, NY], CDT, tag='s_out')
                nc.vector.tensor_sub(s_out[:], qws_b[:], sw[:])
                nc.sync.dma_start(out_s[:, tsl], s_out[:])
                p1 = wpool.tile([NX, TB, NZ, NY], CDT, tag='p1')
                p_out = wpool.tile([NX, TB, NZ, NY], CDT, tag='p_out')
                nc.vector.tensor_add(p1[:], qs_b[:], so[:])
                nc.vector.tensor_add(p_out[:], p1[:], sw[:])
                nc.sync.dma_start(out_p[:, tsl], p_out[:])

    nc.compile()
    _NC_CACHE['nc'] = nc
    return nc


def kernel(pressure, perm, Q, Qw, Time, Pini, Phi, Swini, water_sat):
    import sys
    if '/opt/trn_rl_repo' not in sys.path:
        sys.path.insert(0, '/opt/trn_rl_repo')
    from concourse.bass_utils import run_bass_kernel_spmd

    nc = _build_nc()

    sini = float(np.asarray(Swini[0, 0, 0, 0, 0]))
    mw0 = np.float32((SIGW * sini + BETW) ** 2)
    mo0 = np.float32((SIGO * sini + BETO) ** 2)
    sxT, m1T, idm, nim = _shift_matrices()

    def col(v):
        return np.full((NX, 1), v, np.float32)

    consts = {
        'mw0': col(mw0), 'mo0': col(mo0),
        'cw': col(GSCALE * mw0), 'co': col(GSCALE * mo0),
        'betw': col(BETW), 'beto': col(BETO),
        'wsx': sxT.astype(np.float16), 'wm1': m1T.astype(np.float16),
        'wid': idm.astype(np.float16), 'wni': nim.astype(np.float16),
    }

    def to_xtzy(a, scale=None):  # [T,NZ,NX,NY] -> [NX,T,NZ,NY] fp16 contiguous
        a = np.asarray(a).transpose(2, 0, 1, 3)
        if scale is not None:
            a = a * scale
        return np.ascontiguousarray(a, dtype=np.float16)

    def pad_y(x):  # [NX, ..., NY] -> [NX, ..., NY+4] edge-padded fp16
        shp = x.shape[:-1] + (PW,)
        out = np.zeros(shp, np.float16)
        out[..., 2:2 + NY] = x
        out[..., 1] = x[..., 0]
        out[..., 2 + NY] = x[..., NY - 1]
        return out

    in_maps = []
    for c in range(N_CORES):
        perm_x = to_xtzy(perm[c])
        in_maps.append({
            'press': pad_y(to_xtzy(pressure[c])),
            'perm': perm_x,
            'perm0p': pad_y(perm_x[:, 0]),
            'qs': to_xtzy(Q[c], CQ),
            'qws': to_xtzy(Qw[c], -CQ),
            'sat': to_xtzy(water_sat[c, :T - 1]),
            **consts,
        })

    res = run_bass_kernel_spmd(nc, in_maps, core_ids=list(range(N_CORES)))

    p_loss = np.empty((B, T, NZ, NX, NY), np.float32)
    s_loss = np.empty((B, T, NZ, NX, NY), np.float32)
    for c in range(N_CORES):
        p_loss[c] = res.results[c]['out_p'].astype(np.float32).transpose(1, 2, 0, 3)
        s_loss[c] = res.results[c]['out_s'].astype(np.float32).transpose(1, 2, 0, 3)
    return p_loss, s_loss
